# revision 9
# baseline (speedup 1.0000x reference)
"""2-layer GraphSAGE (PyG SAGEConv, project=True, mean agg) on 8 trn2 NeuronCores.

v3 strategy (graph/data parallel, hardcoded N=50000, E=800000, D=128, 8 cores):
  - Nodes sharded in contiguous ranges of 6250 (padded to 6272 = 49*128) per core.
  - Host sorts edges by (dst core, dst block, src parity, src) and pads each
    (block, parity) class to uniform chunk counts K0/K1 across cores (SPMD).
  - Per layer on device:
      * project own rows transposed: hT = relu(Wp @ x) via PE + scalar engine,
        stored fp8(e4m3) row-table, AllGather -> replicated [50176,128] table.
      * dma_gather (SWDGE) PAIR rows (256B = nodes {2j,2j+1}) with int16 pair
        indices; 4 SWDGE queues give 4 desc-gen CPU pairs in parallel.
      * scatter via one-hot matmuls in fp8 DoubleRow perf mode: one PE
        instruction contracts two 128-edge chunks; one-hots built on DVE in a
        single batched is_equal per destination block.
      * mean via per-dst invdeg multiply, then output matmuls (+relu), with the
        layer-2 projection fused into the layer-1 per-block epilogue.
  - Layer-2 output rows are written per core and concatenated on host.
"""

import math
from contextlib import ExitStack

import numpy as np

import concourse.bacc as bacc
import concourse.bass as bass
import concourse.tile as tile
from concourse import library_config, mybir
from concourse.bass_utils import run_bass_kernel_spmd

P = 128
D = 128
CORES = 8
N_NODES = 50000
N_EDGES = 800000

AF = mybir.ActivationFunctionType
OP = mybir.AluOpType
dt = mybir.dt
NP_BF16 = dt.np(dt.bfloat16)
NP_F8 = dt.np(dt.float8e4)


def _plan(n_nodes, cores):
    nloc = n_nodes // cores
    assert nloc * cores == n_nodes
    nb = math.ceil(nloc / P)
    nloc_pad = nb * P
    npad = cores * nloc_pad
    npair = npad // 2
    assert npair < 32768, "pair idx is int16"
    return nloc, nb, nloc_pad, npad, npair


def preprocess(edge_index, n_nodes, cores):
    """Per-core gather/scatter metadata with parity-pure chunks (K0 even, K1 odd)."""
    nloc, nb, nloc_pad, npad, npair = _plan(n_nodes, cores)
    src = np.asarray(edge_index[0], dtype=np.int64)
    dst = np.asarray(edge_index[1], dtype=np.int64)
    E = src.shape[0]

    deg = np.bincount(dst, minlength=n_nodes).astype(np.float64)
    invdeg = (1.0 / np.maximum(deg, 1.0)).astype(np.float32)

    csrc = src // nloc
    r_src = csrc * nloc_pad + (src - csrc * nloc)  # padded row id of source
    pair = (r_src // 2).astype(np.int64)
    par = (r_src % 2).astype(np.int64)

    cdst = dst // nloc
    ldst = dst - cdst * nloc
    blk = ldst // P
    dblk = ldst % P

    # sort by (dst core, dst block, parity, pair) for DMA locality
    order = np.lexsort((pair, par, blk, cdst))
    s_par = par[order]
    s_pair = pair[order]
    s_dblk = dblk[order]
    key = ((cdst[order] * nb + blk[order]) * 2 + s_par).astype(np.int64)

    counts = np.bincount(key, minlength=cores * nb * 2)
    starts = np.zeros(cores * nb * 2 + 1, dtype=np.int64)
    np.cumsum(counts, out=starts[1:])
    rank = np.arange(E, dtype=np.int64) - starts[key]

    cnt = counts.reshape(cores, nb, 2)
    K0 = max(1, int(math.ceil(cnt[:, :, 0].max() / P)))
    K1 = max(1, int(math.ceil(cnt[:, :, 1].max() / P)))
    KT = K0 + K1

    # slots: evens [0, K0*P), odds [K0*P, KT*P); pad idx 0 / dloc 255
    idx = np.zeros((cores, nb, KT * P), dtype=np.int16)
    dloc = np.full((cores, nb, KT * P), 255.0, dtype=np.float32)

    core_k = key // (nb * 2)
    blk_k = (key // 2) % nb
    m0 = s_par == 0
    m1 = ~m0
    idx[core_k[m0], blk_k[m0], rank[m0]] = s_pair[m0].astype(np.int16)
    idx[core_k[m1], blk_k[m1], K0 * P + rank[m1]] = s_pair[m1].astype(np.int16)
    dloc[core_k[m0], blk_k[m0], rank[m0]] = s_dblk[m0]
    dloc[core_k[m1], blk_k[m1], K0 * P + rank[m1]] = s_dblk[m1]

    def wrap_idx(a):  # flat slots -> [128, N//16] dma_gather layout
        flat = a.reshape(-1)
        w = flat.reshape(-1, 16).T  # [16, N/16]
        return np.tile(w, (8, 1)).copy()

    per_core = []
    for c in range(cores):
        dl = dloc[c].reshape(nb, KT, P).transpose(2, 0, 1).reshape(P, -1)
        inv = np.ones(nloc_pad, dtype=np.float32)
        inv[:nloc] = invdeg[c * nloc : (c + 1) * nloc]
        per_core.append(
            dict(
                idx=wrap_idx(idx[c]),
                dloc=np.ascontiguousarray(dl).astype(NP_BF16),
                invd=np.broadcast_to(inv[None, :], (P, nloc_pad)).astype(NP_BF16).copy(),
            )
        )
    return per_core, K0, K1, invdeg


def build_nc(n_nodes, cores, K0, K1, G, has_bias):
    nloc, nb, nloc_pad, npad, npair = _plan(n_nodes, cores)
    assert nb % G == 0
    ngroups = nb // G
    KT = K0 + K1
    assert (G * KT) % 2 == 0

    BF = dt.bfloat16
    F8 = dt.float8e4
    DR = mybir.MatmulPerfMode.DoubleRow

    nc = bacc.Bacc(
        "TRN2", target_bir_lowering=False, debug=False, num_devices=cores,
        num_swdge_queues=4,
    )

    x16_d = nc.dram_tensor("x16", [nloc_pad, D], BF, kind="ExternalInput").ap()
    idx_d = nc.dram_tensor("idx", [P, nb * KT * P // 16], dt.int16, kind="ExternalInput").ap()
    dloc_d = nc.dram_tensor("dloc", [P, nb * KT], BF, kind="ExternalInput").ap()
    invd_d = nc.dram_tensor("invd", [P, nloc_pad], BF, kind="ExternalInput").ap()
    wdram = {
        n: nc.dram_tensor(n, [P, D], BF, kind="ExternalInput").ap()
        for n in ["Wp1T", "Wl1T", "Wr1T", "Wp2T", "Wl2T", "Wr2T"]
    }
    if has_bias:
        ones_d = nc.dram_tensor("ones1", [1, D], BF, kind="ExternalInput").ap()
        bp1r_d = nc.dram_tensor("bp1r", [1, D], BF, kind="ExternalInput").ap()
        bl1c_d = nc.dram_tensor("bl1c", [P, 1], dt.float32, kind="ExternalInput").ap()
        bp2r_d = nc.dram_tensor("bp2r", [1, D], BF, kind="ExternalInput").ap()
        bl2r_d = nc.dram_tensor("bl2r", [1, D], BF, kind="ExternalInput").ap()
    iota_d = nc.dram_tensor("iota", [P, KT * P], BF, kind="ExternalInput").ap()

    out_own = nc.dram_tensor("out_own", [nloc_pad, D], dt.float32, kind="ExternalOutput").ap()
    h1own = nc.dram_tensor("h1own", [nloc_pad, D], F8).ap()
    h2own = nc.dram_tensor("h2own", [nloc_pad, D], F8).ap()
    table1 = nc.dram_tensor("table1", [npad, D], F8, addr_space="Shared").ap()
    table2 = nc.dram_tensor("table2", [npad, D], F8, addr_space="Shared").ap()

    groups_all = [list(range(cores))]

    with tile.TileContext(nc) as tc, ExitStack() as ctx:
        const = ctx.enter_context(tc.tile_pool(name="const", bufs=1))
        persist = ctx.enter_context(tc.tile_pool(name="persist", bufs=1))
        stage_p = ctx.enter_context(tc.tile_pool(name="stage", bufs=3))
        work = ctx.enter_context(tc.tile_pool(name="work", bufs=3))
        ohp = ctx.enter_context(tc.tile_pool(name="oh", bufs=2))
        aggsb = ctx.enter_context(tc.tile_pool(name="aggsb", bufs=2))
        outp = ctx.enter_context(tc.tile_pool(name="outp", bufs=3))
        psum_agg = ctx.enter_context(tc.tile_pool(name="psum_agg", bufs=4, space="PSUM"))
        psum_mm = ctx.enter_context(tc.tile_pool(name="psum_mm", bufs=3, space="PSUM"))

        nc.gpsimd.load_library(library_config.mlp)

        def cload(ap_dram, shape, dtype, tag):
            t = const.tile(shape, dtype, tag=tag)
            nc.sync.dma_start(t[:], ap_dram)
            return t

        wsb = {n: cload(wdram[n][:, :], [P, D], BF, n) for n in wdram}
        if has_bias:
            ones1 = cload(ones_d[:, :], [1, D], BF, "ones1")
            bp1r = cload(bp1r_d[:, :], [1, D], BF, "bp1r")
            bl1c = cload(bl1c_d[:, :], [P, 1], dt.float32, "bl1c")
            bp2r = cload(bp2r_d[:, :], [1, D], BF, "bp2r")
            bl2r = cload(bl2r_d[:, :], [1, D], BF, "bl2r")
        iota = cload(iota_d[:, :], [P, KT * P], BF, "iota")
        dloc_sb = cload(dloc_d[:, :], [P, nb * KT], BF, "dloc")
        invd_sb = cload(invd_d[:, :], [P, nloc_pad], BF, "invd")
        idx_sb = cload(idx_d[:, :], [P, nb * KT * P // 16], dt.int16, "idx")

        xT_sb = persist.tile([P, nloc_pad], BF, tag="xT")
        h1T_sb = persist.tile([P, nloc_pad], BF, tag="h1T")

        qrot = [1, 2, 3, 0]  # q0 blocks the engine; issue it last per round
        qctr = [0]

        def next_q():
            q = qrot[qctr[0] % 4]
            qctr[0] += 1
            return q

        # ---------------- Phase A: layer-1 projection of own rows ----------
        nc.sync.dma_start_transpose(xT_sb[:, :], x16_d[:, :])
        for b in range(nb):
            sl = slice(b * P, (b + 1) * P)
            p_ps = psum_mm.tile([P, D], dt.float32, tag="mm")
            nc.tensor.matmul(p_ps[:], lhsT=xT_sb[:, sl], rhs=wsb["Wp1T"][:],
                             start=True, stop=not has_bias)
            if has_bias:
                nc.tensor.matmul(p_ps[:], lhsT=ones1[:], rhs=bp1r[:],
                                 start=False, stop=True)
            pr = outp.tile([P, D], F8, tag="pr")
            nc.scalar.activation(pr[:], p_ps[:], AF.Relu)
            nc.sync.dma_start(h1own[sl, :], pr[:])

        nc.gpsimd.collective_compute(
            "AllGather", OP.bypass, replica_groups=groups_all,
            ins=[h1own[:, :]], outs=[table1[:, :]],
        )

        # ---------------- message+aggregate for one layer -------------------
        def agg_layer(table, WlT, WrT, layer):
            tview = table[:, :].rearrange("(a b) d -> a (b d)", b=2)  # [npair, 256]
            half_slots = G * KT * P // 2
            cols = G * KT * P // 16
            for g in range(ngroups):
                st = stage_p.tile([P, G * KT, 2 * D], F8, tag="st")
                nc.gpsimd.dma_gather(
                    st[:, 0 : G * KT // 2, :], tview,
                    idx_sb[:, g * cols : g * cols + cols // 2],
                    half_slots, half_slots, 2 * D, single_packet=False,
                    queue_num=next_q(),
                )
                nc.gpsimd.dma_gather(
                    st[:, G * KT // 2 : G * KT, :], tview,
                    idx_sb[:, g * cols + cols // 2 : (g + 1) * cols],
                    half_slots, half_slots, 2 * D, single_packet=False,
                    queue_num=next_q(),
                )
                for bb in range(G):
                    b = g * G + bb
                    sl = slice(b * P, (b + 1) * P)
                    oh = ohp.tile([P, KT, P], F8, tag="oh")
                    nc.vector.tensor_tensor(
                        out=oh[:],
                        in0=dloc_sb[:, b * KT : (b + 1) * KT].unsqueeze(2).to_broadcast([P, KT, P]),
                        in1=iota[:].rearrange("p (k q) -> p k q", k=KT),
                        op=OP.is_equal,
                    )
                    # fp8 DoubleRow over consecutive chunk pairs; the parity
                    # bridge pair (last even, first odd) uses a 384-elem stride
                    base_ap = st[:]
                    pstride = base_ap.ap[0][0]

                    def chunk_addr(i):
                        return (bb * KT + i) * 2 * D + (0 if i < K0 else D)

                    def pair_lhsT(i):
                        stride = chunk_addr(i + 1) - chunk_addr(i)
                        return bass.AP(
                            base_ap.tensor,
                            base_ap.offset + chunk_addr(i),
                            [[pstride, P], [stride, 2], [1, D]],
                        )

                    ops = []
                    t = 0
                    while t + 2 <= KT:
                        ops.append((pair_lhsT(t), oh[:, t : t + 2, :], DR))
                        t += 2
                    if t < KT:
                        par = 0 if t < K0 else 1
                        ops.append((
                            st[:, bb * KT + t, par * D : (par + 1) * D],
                            oh[:, t, :],
                            None,
                        ))
                    agg_ps = psum_agg.tile([P, P], dt.float32)
                    for i, (l, r, pm) in enumerate(ops):
                        nc.tensor.matmul(
                            agg_ps[:], lhsT=l, rhs=r,
                            start=(i == 0), stop=(i == len(ops) - 1),
                            perf_mode=pm,
                        )
                    aggT = aggsb.tile([P, P], BF)
                    nc.vector.tensor_tensor(
                        out=aggT[:], in0=agg_ps[:], in1=invd_sb[:, sl], op=OP.mult
                    )
                    if layer == 1:
                        o_ps = psum_mm.tile([P, P], dt.float32, tag="mm")
                        nc.tensor.matmul(o_ps[:], lhsT=WlT[:], rhs=aggT[:], start=True, stop=False)
                        nc.tensor.matmul(o_ps[:], lhsT=WrT[:], rhs=xT_sb[:, sl], start=False, stop=True)
                        if has_bias:
                            nc.scalar.activation(h1T_sb[:, sl], o_ps[:], AF.Relu, bias=bl1c[:], scale=1.0)
                        else:
                            nc.scalar.activation(h1T_sb[:, sl], o_ps[:], AF.Relu)
                        # fused layer-2 projection of this block
                        p_ps = psum_mm.tile([P, D], dt.float32, tag="mm")
                        nc.tensor.matmul(p_ps[:], lhsT=h1T_sb[:, sl], rhs=wsb["Wp2T"][:],
                                         start=True, stop=not has_bias)
                        if has_bias:
                            nc.tensor.matmul(p_ps[:], lhsT=ones1[:], rhs=bp2r[:],
                                             start=False, stop=True)
                        pr = outp.tile([P, D], F8, tag="pr")
                        nc.scalar.activation(pr[:], p_ps[:], AF.Relu)
                        nc.sync.dma_start(h2own[sl, :], pr[:])
                    else:
                        o_ps = psum_mm.tile([P, D], dt.float32, tag="mm")
                        nc.tensor.matmul(o_ps[:], lhsT=aggT[:], rhs=WlT[:], start=True, stop=False)
                        nc.tensor.matmul(o_ps[:], lhsT=h1T_sb[:, sl], rhs=WrT[:],
                                         start=False, stop=not has_bias)
                        if has_bias:
                            nc.tensor.matmul(o_ps[:], lhsT=ones1[:], rhs=bl2r[:],
                                             start=False, stop=True)
                        ob = outp.tile([P, D], dt.float32, tag="ob")
                        nc.scalar.activation(ob[:], o_ps[:], AF.Copy)
                        nc.sync.dma_start(out_own[sl, :], ob[:])

        agg_layer(table1, wsb["Wl1T"], wsb["Wr1T"], layer=1)

        nc.gpsimd.collective_compute(
            "AllGather", OP.bypass, replica_groups=groups_all,
            ins=[h2own[:, :]], outs=[table2[:, :]],
        )

        agg_layer(table2, wsb["Wl2T"], wsb["Wr2T"], layer=2)

    nc.compile()
    return nc


def make_in_maps(inputs, per_core, n_nodes, cores, K0, K1, has_bias):
    nloc, nb, nloc_pad, npad, npair = _plan(n_nodes, cores)
    KT = K0 + K1
    x = np.asarray(inputs["x"], dtype=np.float32)
    consts = dict(
        Wp1T=np.asarray(inputs["Wp1"]).T.astype(NP_BF16),
        Wl1T=np.asarray(inputs["Wl1"]).T.astype(NP_BF16),
        Wr1T=np.asarray(inputs["Wr1"]).T.astype(NP_BF16),
        Wp2T=np.asarray(inputs["Wp2"]).T.astype(NP_BF16),
        Wl2T=np.asarray(inputs["Wl2"]).T.astype(NP_BF16),
        Wr2T=np.asarray(inputs["Wr2"]).T.astype(NP_BF16),
        iota=np.tile(np.arange(P, dtype=np.float32)[None, :], (P, KT)).astype(NP_BF16),
    )
    if has_bias:
        consts.update(
            ones1=np.ones((1, D), dtype=np.float32).astype(NP_BF16),
            bp1r=np.asarray(inputs["bp1"], np.float32).reshape(1, D).astype(NP_BF16),
            bl1c=np.asarray(inputs["bl1"], np.float32).reshape(P, 1).copy(),
            bp2r=np.asarray(inputs["bp2"], np.float32).reshape(1, D).astype(NP_BF16),
            bl2r=np.asarray(inputs["bl2"], np.float32).reshape(1, D).astype(NP_BF16),
        )
    in_maps = []
    for c in range(cores):
        xo = np.zeros((nloc_pad, D), dtype=np.float32)
        xo[:nloc] = x[c * nloc : (c + 1) * nloc]
        m = dict(consts)
        m["x16"] = xo.astype(NP_BF16)
        m.update(per_core[c])
        in_maps.append(m)
    return in_maps


_BUILT = {}


def _run(inputs, n_nodes, n_edges, cores, G=7, trace=False):
    per_core, K0, K1, _ = preprocess(inputs["edge_index"], n_nodes, cores)
    has_bias = any(
        np.any(np.asarray(inputs[k]) != 0) for k in ["bp1", "bl1", "bp2", "bl2"]
    )
    key = (n_nodes, cores, K0, K1, G, has_bias)
    if key not in _BUILT:
        _BUILT[key] = build_nc(n_nodes, cores, K0, K1, G, has_bias)
    nc = _BUILT[key]
    in_maps = make_in_maps(inputs, per_core, n_nodes, cores, K0, K1, has_bias)
    res = run_bass_kernel_spmd(nc, in_maps, list(range(cores)), trace=trace)
    nloc, nb, nloc_pad, npad, npair = _plan(n_nodes, cores)
    out = np.concatenate([res.results[c]["out_own"][:nloc] for c in range(cores)], axis=0)
    return out.astype(np.float32), res


def kernel(**inputs):
    out, _ = _run(inputs, N_NODES, N_EDGES, CORES, G=7)
    return out


# revision 10
# speedup vs baseline: 1.0627x; 1.0627x over previous
"""2-layer GraphSAGE (PyG SAGEConv, project=True, mean agg) on 8 trn2 NeuronCores.

v3 strategy (graph/data parallel, hardcoded N=50000, E=800000, D=128, 8 cores):
  - Nodes sharded in contiguous ranges of 6250 (padded to 6272 = 49*128) per core.
  - Host sorts edges by (dst core, dst block, src parity, src) and pads each
    (block, parity) class to uniform chunk counts K0/K1 across cores (SPMD).
  - Per layer on device:
      * project own rows transposed: hT = relu(Wp @ x) via PE + scalar engine,
        stored fp8(e4m3) row-table, AllGather -> replicated [50176,128] table.
      * dma_gather (SWDGE) PAIR rows (256B = nodes {2j,2j+1}) with int16 pair
        indices; 4 SWDGE queues give 4 desc-gen CPU pairs in parallel.
      * scatter via one-hot matmuls in fp8 DoubleRow perf mode: one PE
        instruction contracts two 128-edge chunks; one-hots built on DVE in a
        single batched is_equal per destination block.
      * mean via per-dst invdeg multiply, then output matmuls (+relu), with the
        layer-2 projection fused into the layer-1 per-block epilogue.
  - Layer-2 output rows are written per core and concatenated on host.
"""

import math
from contextlib import ExitStack

import numpy as np

import concourse.bacc as bacc
import concourse.bass as bass
import concourse.tile as tile
from concourse import library_config, mybir
from concourse.bass_utils import run_bass_kernel_spmd

P = 128
D = 128
CORES = 8
N_NODES = 50000
N_EDGES = 800000

AF = mybir.ActivationFunctionType
OP = mybir.AluOpType
dt = mybir.dt
NP_BF16 = dt.np(dt.bfloat16)
NP_F8 = dt.np(dt.float8e4)


def _plan(n_nodes, cores):
    nloc = n_nodes // cores
    assert nloc * cores == n_nodes
    nb = math.ceil(nloc / P)
    nloc_pad = nb * P
    npad = cores * nloc_pad
    npair = npad // 2
    assert npair < 32768, "pair idx is int16"
    return nloc, nb, nloc_pad, npad, npair


def preprocess(edge_index, n_nodes, cores):
    """Per-core gather/scatter metadata with parity-pure chunks (K0 even, K1 odd)."""
    nloc, nb, nloc_pad, npad, npair = _plan(n_nodes, cores)
    src = np.asarray(edge_index[0], dtype=np.int64)
    dst = np.asarray(edge_index[1], dtype=np.int64)
    E = src.shape[0]

    deg = np.bincount(dst, minlength=n_nodes).astype(np.float64)
    invdeg = (1.0 / np.maximum(deg, 1.0)).astype(np.float32)

    csrc = src // nloc
    r_src = csrc * nloc_pad + (src - csrc * nloc)  # padded row id of source
    pair = (r_src // 2).astype(np.int64)
    par = (r_src % 2).astype(np.int64)

    cdst = dst // nloc
    ldst = dst - cdst * nloc
    blk = ldst // P
    dblk = ldst % P

    # sort by (dst core, dst block, parity, pair) for DMA locality
    order = np.lexsort((pair, par, blk, cdst))
    s_par = par[order]
    s_pair = pair[order]
    s_dblk = dblk[order]
    key = ((cdst[order] * nb + blk[order]) * 2 + s_par).astype(np.int64)

    counts = np.bincount(key, minlength=cores * nb * 2)
    starts = np.zeros(cores * nb * 2 + 1, dtype=np.int64)
    np.cumsum(counts, out=starts[1:])
    rank = np.arange(E, dtype=np.int64) - starts[key]

    cnt = counts.reshape(cores, nb, 2)
    K0 = max(1, int(math.ceil(cnt[:, :, 0].max() / P)))
    K1 = max(1, int(math.ceil(cnt[:, :, 1].max() / P)))
    KT = K0 + K1

    # slots: evens [0, K0*P), odds [K0*P, KT*P); pad idx 0 / dloc 255
    idx = np.zeros((cores, nb, KT * P), dtype=np.int16)
    dloc = np.full((cores, nb, KT * P), 255, dtype=np.int32)

    core_k = key // (nb * 2)
    blk_k = (key // 2) % nb
    m0 = s_par == 0
    m1 = ~m0
    idx[core_k[m0], blk_k[m0], rank[m0]] = s_pair[m0].astype(np.int16)
    idx[core_k[m1], blk_k[m1], K0 * P + rank[m1]] = s_pair[m1].astype(np.int16)
    dloc[core_k[m0], blk_k[m0], rank[m0]] = s_dblk[m0]
    dloc[core_k[m1], blk_k[m1], K0 * P + rank[m1]] = s_dblk[m1]

    def wrap_idx(a):  # flat slots -> [128, N//16] dma_gather layout
        flat = a.reshape(-1)
        w = flat.reshape(-1, 16).T  # [16, N/16]
        return np.tile(w, (8, 1)).copy()

    per_core = []
    for c in range(cores):
        dl = dloc[c].reshape(nb, KT, P).transpose(2, 0, 1).reshape(P, -1)
        inv = np.ones(nloc_pad, dtype=np.float32)
        inv[:nloc] = invdeg[c * nloc : (c + 1) * nloc]
        per_core.append(
            dict(
                idx=wrap_idx(idx[c]),
                dloc=np.ascontiguousarray(dl).astype(np.uint8),
                invd=np.broadcast_to(inv[None, :], (P, nloc_pad)).astype(NP_BF16).copy(),
            )
        )
    return per_core, K0, K1, invdeg


def build_nc(n_nodes, cores, K0, K1, G, has_bias):
    nloc, nb, nloc_pad, npad, npair = _plan(n_nodes, cores)
    assert nb % G == 0
    ngroups = nb // G
    KT = K0 + K1
    assert (G * KT) % 2 == 0

    BF = dt.bfloat16
    F8 = dt.float8e4
    DR = mybir.MatmulPerfMode.DoubleRow

    nc = bacc.Bacc(
        "TRN2", target_bir_lowering=False, debug=False, num_devices=cores,
        num_swdge_queues=4,
    )

    x16_d = nc.dram_tensor("x16", [nloc_pad, D], BF, kind="ExternalInput").ap()
    idx_d = nc.dram_tensor("idx", [P, nb * KT * P // 16], dt.int16, kind="ExternalInput").ap()
    dloc_d = nc.dram_tensor("dloc", [P, nb * KT], dt.uint8, kind="ExternalInput").ap()
    invd_d = nc.dram_tensor("invd", [P, nloc_pad], BF, kind="ExternalInput").ap()
    wdram = {
        n: nc.dram_tensor(n, [P, D], BF, kind="ExternalInput").ap()
        for n in ["Wp1T", "Wl1T", "Wr1T", "Wp2T", "Wl2T", "Wr2T"]
    }
    if has_bias:
        ones_d = nc.dram_tensor("ones1", [1, D], BF, kind="ExternalInput").ap()
        bp1r_d = nc.dram_tensor("bp1r", [1, D], BF, kind="ExternalInput").ap()
        bl1c_d = nc.dram_tensor("bl1c", [P, 1], dt.float32, kind="ExternalInput").ap()
        bp2r_d = nc.dram_tensor("bp2r", [1, D], BF, kind="ExternalInput").ap()
        bl2r_d = nc.dram_tensor("bl2r", [1, D], BF, kind="ExternalInput").ap()
    iota_d = nc.dram_tensor("iota", [P, KT * P], dt.uint8, kind="ExternalInput").ap()

    out_own = nc.dram_tensor("out_own", [nloc_pad, D], dt.float32, kind="ExternalOutput").ap()
    h1own = nc.dram_tensor("h1own", [nloc_pad, D], F8).ap()
    h2own = nc.dram_tensor("h2own", [nloc_pad, D], F8).ap()
    table1 = nc.dram_tensor("table1", [npad, D], F8, addr_space="Shared").ap()
    table2 = nc.dram_tensor("table2", [npad, D], F8, addr_space="Shared").ap()

    groups_all = [list(range(cores))]

    with tile.TileContext(nc) as tc, ExitStack() as ctx:
        const = ctx.enter_context(tc.tile_pool(name="const", bufs=1))
        persist = ctx.enter_context(tc.tile_pool(name="persist", bufs=1))
        stage_p = ctx.enter_context(tc.tile_pool(name="stage", bufs=4))
        work = ctx.enter_context(tc.tile_pool(name="work", bufs=3))
        ohp = ctx.enter_context(tc.tile_pool(name="oh", bufs=2))
        aggsb = ctx.enter_context(tc.tile_pool(name="aggsb", bufs=2))
        outp = ctx.enter_context(tc.tile_pool(name="outp", bufs=3))
        psum_agg = ctx.enter_context(tc.tile_pool(name="psum_agg", bufs=4, space="PSUM"))
        psum_mm = ctx.enter_context(tc.tile_pool(name="psum_mm", bufs=3, space="PSUM"))

        nc.gpsimd.load_library(library_config.mlp)

        def cload(ap_dram, shape, dtype, tag):
            t = const.tile(shape, dtype, tag=tag)
            nc.sync.dma_start(t[:], ap_dram)
            return t

        wsb = {n: cload(wdram[n][:, :], [P, D], BF, n) for n in wdram}
        if has_bias:
            ones1 = cload(ones_d[:, :], [1, D], BF, "ones1")
            bp1r = cload(bp1r_d[:, :], [1, D], BF, "bp1r")
            bl1c = cload(bl1c_d[:, :], [P, 1], dt.float32, "bl1c")
            bp2r = cload(bp2r_d[:, :], [1, D], BF, "bp2r")
            bl2r = cload(bl2r_d[:, :], [1, D], BF, "bl2r")
        iota = cload(iota_d[:, :], [P, KT * P], dt.uint8, "iota")
        dloc_sb = cload(dloc_d[:, :], [P, nb * KT], dt.uint8, "dloc")
        invd_sb = cload(invd_d[:, :], [P, nloc_pad], BF, "invd")
        idx_sb = cload(idx_d[:, :], [P, nb * KT * P // 16], dt.int16, "idx")

        xT_sb = persist.tile([P, nloc_pad], BF, tag="xT")
        h1T_sb = persist.tile([P, nloc_pad], BF, tag="h1T")

        qrot = [1, 2, 3, 0]  # q0 blocks the engine; issue it last per round
        qctr = [0]

        def next_q():
            q = qrot[qctr[0] % 4]
            qctr[0] += 1
            return q

        # ---------------- Phase A: layer-1 projection of own rows ----------
        nc.sync.dma_start_transpose(xT_sb[:, :], x16_d[:, :])
        for b in range(nb):
            sl = slice(b * P, (b + 1) * P)
            p_ps = psum_mm.tile([P, D], dt.float32, tag="mm")
            nc.tensor.matmul(p_ps[:], lhsT=xT_sb[:, sl], rhs=wsb["Wp1T"][:],
                             start=True, stop=not has_bias)
            if has_bias:
                nc.tensor.matmul(p_ps[:], lhsT=ones1[:], rhs=bp1r[:],
                                 start=False, stop=True)
            pr = outp.tile([P, D], F8, tag="pr")
            nc.scalar.activation(pr[:], p_ps[:], AF.Relu)
            nc.sync.dma_start(h1own[sl, :], pr[:])

        nc.gpsimd.collective_compute(
            "AllGather", OP.bypass, replica_groups=groups_all,
            ins=[h1own[:, :]], outs=[table1[:, :]],
        )

        # ---------------- message+aggregate for one layer -------------------
        def agg_layer(table, WlT, WrT, layer):
            tview = table[:, :].rearrange("(a b) d -> a (b d)", b=2)  # [npair, 256]
            half_slots = G * KT * P // 2
            cols = G * KT * P // 16
            for g in range(ngroups):
                st = stage_p.tile([P, G * KT, 2 * D], F8, tag="st")
                nc.gpsimd.dma_gather(
                    st[:, 0 : G * KT // 2, :], tview,
                    idx_sb[:, g * cols : g * cols + cols // 2],
                    half_slots, half_slots, 2 * D, single_packet=False,
                    queue_num=next_q(),
                )
                nc.gpsimd.dma_gather(
                    st[:, G * KT // 2 : G * KT, :], tview,
                    idx_sb[:, g * cols + cols // 2 : (g + 1) * cols],
                    half_slots, half_slots, 2 * D, single_packet=False,
                    queue_num=next_q(),
                )
                for bb in range(G):
                    b = g * G + bb
                    sl = slice(b * P, (b + 1) * P)
                    oh = ohp.tile([P, KT, P], F8, tag="oh")
                    nc.vector.tensor_tensor(
                        out=oh[:],
                        in0=dloc_sb[:, b * KT : (b + 1) * KT].unsqueeze(2).to_broadcast([P, KT, P]),
                        in1=iota[:].rearrange("p (k q) -> p k q", k=KT),
                        op=OP.is_equal,
                    )
                    # fp8 DoubleRow over consecutive chunk pairs; the parity
                    # bridge pair (last even, first odd) uses a 384-elem stride
                    base_ap = st[:]
                    pstride = base_ap.ap[0][0]

                    def chunk_addr(i):
                        return (bb * KT + i) * 2 * D + (0 if i < K0 else D)

                    def pair_lhsT(i):
                        stride = chunk_addr(i + 1) - chunk_addr(i)
                        return bass.AP(
                            base_ap.tensor,
                            base_ap.offset + chunk_addr(i),
                            [[pstride, P], [stride, 2], [1, D]],
                        )

                    ops = []
                    t = 0
                    while t + 2 <= KT:
                        ops.append((pair_lhsT(t), oh[:, t : t + 2, :], DR))
                        t += 2
                    if t < KT:
                        par = 0 if t < K0 else 1
                        ops.append((
                            st[:, bb * KT + t, par * D : (par + 1) * D],
                            oh[:, t, :],
                            None,
                        ))
                    agg_ps = psum_agg.tile([P, P], dt.float32)
                    for i, (l, r, pm) in enumerate(ops):
                        nc.tensor.matmul(
                            agg_ps[:], lhsT=l, rhs=r,
                            start=(i == 0), stop=(i == len(ops) - 1),
                            perf_mode=pm,
                        )
                    aggT = aggsb.tile([P, P], BF)
                    nc.vector.tensor_tensor(
                        out=aggT[:], in0=agg_ps[:], in1=invd_sb[:, sl], op=OP.mult
                    )
                    if layer == 1:
                        o_ps = psum_mm.tile([P, P], dt.float32, tag="mm")
                        nc.tensor.matmul(o_ps[:], lhsT=WlT[:], rhs=aggT[:], start=True, stop=False)
                        nc.tensor.matmul(o_ps[:], lhsT=WrT[:], rhs=xT_sb[:, sl], start=False, stop=True)
                        if has_bias:
                            nc.scalar.activation(h1T_sb[:, sl], o_ps[:], AF.Relu, bias=bl1c[:], scale=1.0)
                        else:
                            nc.scalar.activation(h1T_sb[:, sl], o_ps[:], AF.Relu)
                        # fused layer-2 projection of this block
                        p_ps = psum_mm.tile([P, D], dt.float32, tag="mm")
                        nc.tensor.matmul(p_ps[:], lhsT=h1T_sb[:, sl], rhs=wsb["Wp2T"][:],
                                         start=True, stop=not has_bias)
                        if has_bias:
                            nc.tensor.matmul(p_ps[:], lhsT=ones1[:], rhs=bp2r[:],
                                             start=False, stop=True)
                        pr = outp.tile([P, D], F8, tag="pr")
                        nc.scalar.activation(pr[:], p_ps[:], AF.Relu)
                        nc.sync.dma_start(h2own[sl, :], pr[:])
                    else:
                        o_ps = psum_mm.tile([P, D], dt.float32, tag="mm")
                        nc.tensor.matmul(o_ps[:], lhsT=aggT[:], rhs=WlT[:], start=True, stop=False)
                        nc.tensor.matmul(o_ps[:], lhsT=h1T_sb[:, sl], rhs=WrT[:],
                                         start=False, stop=not has_bias)
                        if has_bias:
                            nc.tensor.matmul(o_ps[:], lhsT=ones1[:], rhs=bl2r[:],
                                             start=False, stop=True)
                        ob = outp.tile([P, D], dt.float32, tag="ob")
                        nc.scalar.activation(ob[:], o_ps[:], AF.Copy)
                        nc.sync.dma_start(out_own[sl, :], ob[:])

        agg_layer(table1, wsb["Wl1T"], wsb["Wr1T"], layer=1)

        nc.gpsimd.collective_compute(
            "AllGather", OP.bypass, replica_groups=groups_all,
            ins=[h2own[:, :]], outs=[table2[:, :]],
        )

        agg_layer(table2, wsb["Wl2T"], wsb["Wr2T"], layer=2)

    nc.compile()
    return nc


def make_in_maps(inputs, per_core, n_nodes, cores, K0, K1, has_bias):
    nloc, nb, nloc_pad, npad, npair = _plan(n_nodes, cores)
    KT = K0 + K1
    x = np.asarray(inputs["x"], dtype=np.float32)
    consts = dict(
        Wp1T=np.asarray(inputs["Wp1"]).T.astype(NP_BF16),
        Wl1T=np.asarray(inputs["Wl1"]).T.astype(NP_BF16),
        Wr1T=np.asarray(inputs["Wr1"]).T.astype(NP_BF16),
        Wp2T=np.asarray(inputs["Wp2"]).T.astype(NP_BF16),
        Wl2T=np.asarray(inputs["Wl2"]).T.astype(NP_BF16),
        Wr2T=np.asarray(inputs["Wr2"]).T.astype(NP_BF16),
        iota=np.tile(np.arange(P, dtype=np.uint8)[None, :], (P, KT)).copy(),
    )
    if has_bias:
        consts.update(
            ones1=np.ones((1, D), dtype=np.float32).astype(NP_BF16),
            bp1r=np.asarray(inputs["bp1"], np.float32).reshape(1, D).astype(NP_BF16),
            bl1c=np.asarray(inputs["bl1"], np.float32).reshape(P, 1).copy(),
            bp2r=np.asarray(inputs["bp2"], np.float32).reshape(1, D).astype(NP_BF16),
            bl2r=np.asarray(inputs["bl2"], np.float32).reshape(1, D).astype(NP_BF16),
        )
    in_maps = []
    for c in range(cores):
        xo = np.zeros((nloc_pad, D), dtype=np.float32)
        xo[:nloc] = x[c * nloc : (c + 1) * nloc]
        m = dict(consts)
        m["x16"] = xo.astype(NP_BF16)
        m.update(per_core[c])
        in_maps.append(m)
    return in_maps


_BUILT = {}


def _run(inputs, n_nodes, n_edges, cores, G=7, trace=False):
    per_core, K0, K1, _ = preprocess(inputs["edge_index"], n_nodes, cores)
    has_bias = any(
        np.any(np.asarray(inputs[k]) != 0) for k in ["bp1", "bl1", "bp2", "bl2"]
    )
    key = (n_nodes, cores, K0, K1, G, has_bias)
    if key not in _BUILT:
        _BUILT[key] = build_nc(n_nodes, cores, K0, K1, G, has_bias)
    nc = _BUILT[key]
    in_maps = make_in_maps(inputs, per_core, n_nodes, cores, K0, K1, has_bias)
    res = run_bass_kernel_spmd(nc, in_maps, list(range(cores)), trace=trace)
    nloc, nb, nloc_pad, npad, npair = _plan(n_nodes, cores)
    out = np.concatenate([res.results[c]["out_own"][:nloc] for c in range(cores)], axis=0)
    return out.astype(np.float32), res


def kernel(**inputs):
    out, _ = _run(inputs, N_NODES, N_EDGES, CORES, G=7)
    return out


# revision 13
# speedup vs baseline: 1.1347x; 1.0678x over previous
"""2-layer GraphSAGE (PyG SAGEConv, project=True, mean agg) on 8 trn2 NeuronCores.

v3 strategy (graph/data parallel, hardcoded N=50000, E=800000, D=128, 8 cores):
  - Nodes sharded in contiguous ranges of 6250 (padded to 6272 = 49*128) per core.
  - Host sorts edges by (dst core, dst block, src parity, src) and pads each
    (block, parity) class to uniform chunk counts K0/K1 across cores (SPMD).
  - Per layer on device:
      * project own rows transposed: hT = relu(Wp @ x) via PE + scalar engine,
        stored fp8(e4m3) row-table, AllGather -> replicated [50176,128] table.
      * dma_gather (SWDGE) PAIR rows (256B = nodes {2j,2j+1}) with int16 pair
        indices; 4 SWDGE queues give 4 desc-gen CPU pairs in parallel.
      * scatter via one-hot matmuls in fp8 DoubleRow perf mode: one PE
        instruction contracts two 128-edge chunks; one-hots built on DVE in a
        single batched is_equal per destination block.
      * mean via per-dst invdeg multiply, then output matmuls (+relu), with the
        layer-2 projection fused into the layer-1 per-block epilogue.
  - Layer-2 output rows are written per core and concatenated on host.
"""

import math
from contextlib import ExitStack

import numpy as np

import concourse.bacc as bacc
import concourse.bass as bass
import concourse.tile as tile
from concourse import library_config, mybir
from concourse.bass_utils import run_bass_kernel_spmd

P = 128
D = 128
CORES = 8
N_NODES = 50000
N_EDGES = 800000

AF = mybir.ActivationFunctionType
OP = mybir.AluOpType
dt = mybir.dt
NP_BF16 = dt.np(dt.bfloat16)
NP_F8 = dt.np(dt.float8e4)


def _plan(n_nodes, cores):
    nloc = n_nodes // cores
    assert nloc * cores == n_nodes
    nb = math.ceil(nloc / P)
    nloc_pad = nb * P
    npad = cores * nloc_pad
    npair = npad // 2
    assert npair < 32768, "pair idx is int16"
    return nloc, nb, nloc_pad, npad, npair


def preprocess(edge_index, n_nodes, cores):
    """Per-core gather/scatter metadata with parity-pure chunks (K0 even, K1 odd)."""
    nloc, nb, nloc_pad, npad, npair = _plan(n_nodes, cores)
    src = np.asarray(edge_index[0], dtype=np.int64)
    dst = np.asarray(edge_index[1], dtype=np.int64)
    E = src.shape[0]

    deg = np.bincount(dst, minlength=n_nodes).astype(np.float64)
    invdeg = (1.0 / np.maximum(deg, 1.0)).astype(np.float32)

    csrc = src // nloc
    r_src = csrc * nloc_pad + (src - csrc * nloc)  # padded row id of source
    pair = (r_src // 2).astype(np.int64)
    par = (r_src % 2).astype(np.int64)

    cdst = dst // nloc
    ldst = dst - cdst * nloc
    blk = ldst // P
    dblk = ldst % P

    # sort by (dst core, dst block, parity, pair) for DMA locality
    order = np.lexsort((pair, par, blk, cdst))
    s_par = par[order]
    s_pair = pair[order]
    s_dblk = dblk[order]
    key = ((cdst[order] * nb + blk[order]) * 2 + s_par).astype(np.int64)

    counts = np.bincount(key, minlength=cores * nb * 2)
    starts = np.zeros(cores * nb * 2 + 1, dtype=np.int64)
    np.cumsum(counts, out=starts[1:])
    rank = np.arange(E, dtype=np.int64) - starts[key]

    cnt = counts.reshape(cores, nb, 2)
    K0 = max(1, int(math.ceil(cnt[:, :, 0].max() / P)))
    K1 = max(1, int(math.ceil(cnt[:, :, 1].max() / P)))
    KT = K0 + K1

    # slots: evens [0, K0*P), odds [K0*P, KT*P); pad idx 0 / dloc 255
    idx = np.zeros((cores, nb, KT * P), dtype=np.int16)
    dloc = np.full((cores, nb, KT * P), 255, dtype=np.int32)

    core_k = key // (nb * 2)
    blk_k = (key // 2) % nb
    m0 = s_par == 0
    m1 = ~m0
    idx[core_k[m0], blk_k[m0], rank[m0]] = s_pair[m0].astype(np.int16)
    idx[core_k[m1], blk_k[m1], K0 * P + rank[m1]] = s_pair[m1].astype(np.int16)
    dloc[core_k[m0], blk_k[m0], rank[m0]] = s_dblk[m0]
    dloc[core_k[m1], blk_k[m1], K0 * P + rank[m1]] = s_dblk[m1]

    def wrap_idx(a):  # flat slots -> [128, N//16] dma_gather layout
        flat = a.reshape(-1)
        w = flat.reshape(-1, 16).T  # [16, N/16]
        return np.tile(w, (8, 1)).copy()

    per_core = []
    for c in range(cores):
        dl = dloc[c].reshape(nb, KT, P).transpose(2, 0, 1).reshape(P, -1)
        inv = np.ones(nloc_pad, dtype=np.float32)
        inv[:nloc] = invdeg[c * nloc : (c + 1) * nloc]
        per_core.append(
            dict(
                idx=wrap_idx(idx[c]),
                dloc=np.ascontiguousarray(dl).astype(np.uint8),
                invd=np.broadcast_to(inv[None, :], (P, nloc_pad)).astype(NP_BF16).copy(),
            )
        )
    return per_core, K0, K1, invdeg


def build_nc(n_nodes, cores, K0, K1, G, has_bias):
    nloc, nb, nloc_pad, npad, npair = _plan(n_nodes, cores)
    assert nb % G == 0
    ngroups = nb // G
    KT = K0 + K1
    assert (G * KT) % 2 == 0

    BF = dt.bfloat16
    F8 = dt.float8e4
    DR = mybir.MatmulPerfMode.DoubleRow

    nc = bacc.Bacc(
        "TRN2", target_bir_lowering=False, debug=False, num_devices=cores,
        num_swdge_queues=4,
    )

    x16_d = nc.dram_tensor("x16", [nloc_pad, D], BF, kind="ExternalInput").ap()
    idx_d = nc.dram_tensor("idx", [P, nb * KT * P // 16], dt.int16, kind="ExternalInput").ap()
    dloc_d = nc.dram_tensor("dloc", [P, nb * KT], dt.uint8, kind="ExternalInput").ap()
    invd_d = nc.dram_tensor("invd", [P, nloc_pad], BF, kind="ExternalInput").ap()
    wdram = {
        n: nc.dram_tensor(n, [P, D], BF, kind="ExternalInput").ap()
        for n in ["Wp1T", "Wl1T", "Wr1T", "Wp2T", "Wl2T", "Wr2T"]
    }
    if has_bias:
        ones_d = nc.dram_tensor("ones1", [1, D], BF, kind="ExternalInput").ap()
        bp1r_d = nc.dram_tensor("bp1r", [1, D], BF, kind="ExternalInput").ap()
        bl1c_d = nc.dram_tensor("bl1c", [P, 1], dt.float32, kind="ExternalInput").ap()
        bp2r_d = nc.dram_tensor("bp2r", [1, D], BF, kind="ExternalInput").ap()
        bl2r_d = nc.dram_tensor("bl2r", [1, D], BF, kind="ExternalInput").ap()
    iota_d = nc.dram_tensor("iota", [P, KT * P], dt.uint8, kind="ExternalInput").ap()

    out_own = nc.dram_tensor("out_own", [nloc_pad, D], dt.float32, kind="ExternalOutput").ap()
    h1own = nc.dram_tensor("h1own", [nloc_pad, D], F8).ap()
    h2own = nc.dram_tensor("h2own", [nloc_pad, D], F8).ap()
    table1 = nc.dram_tensor("table1", [npad, D], F8, addr_space="Shared").ap()
    table2 = nc.dram_tensor("table2", [npad, D], F8, addr_space="Shared").ap()

    groups_all = [list(range(cores))]

    with tile.TileContext(nc) as tc, ExitStack() as ctx:
        const = ctx.enter_context(tc.tile_pool(name="const", bufs=1))
        persist = ctx.enter_context(tc.tile_pool(name="persist", bufs=1))
        stage_p = ctx.enter_context(tc.tile_pool(name="stage", bufs=4))
        work = ctx.enter_context(tc.tile_pool(name="work", bufs=3))
        ohp = ctx.enter_context(tc.tile_pool(name="oh", bufs=2))
        aggsb = ctx.enter_context(tc.tile_pool(name="aggsb", bufs=2))
        outp = ctx.enter_context(tc.tile_pool(name="outp", bufs=3))
        psum_agg = ctx.enter_context(tc.tile_pool(name="psum_agg", bufs=4, space="PSUM"))
        psum_mm = ctx.enter_context(tc.tile_pool(name="psum_mm", bufs=3, space="PSUM"))

        nc.gpsimd.load_library(library_config.mlp)

        def cload(ap_dram, shape, dtype, tag):
            t = const.tile(shape, dtype, tag=tag)
            nc.sync.dma_start(t[:], ap_dram)
            return t

        wsb = {n: cload(wdram[n][:, :], [P, D], BF, n) for n in wdram}
        if has_bias:
            ones1 = cload(ones_d[:, :], [1, D], BF, "ones1")
            bp1r = cload(bp1r_d[:, :], [1, D], BF, "bp1r")
            bl1c = cload(bl1c_d[:, :], [P, 1], dt.float32, "bl1c")
            bp2r = cload(bp2r_d[:, :], [1, D], BF, "bp2r")
            bl2r = cload(bl2r_d[:, :], [1, D], BF, "bl2r")
        iota = cload(iota_d[:, :], [P, KT * P], dt.uint8, "iota")
        dloc_sb = cload(dloc_d[:, :], [P, nb * KT], dt.uint8, "dloc")
        invd_sb = cload(invd_d[:, :], [P, nloc_pad], BF, "invd")
        idx_sb = cload(idx_d[:, :], [P, nb * KT * P // 16], dt.int16, "idx")

        xT_sb = persist.tile([P, nloc_pad], BF, tag="xT")
        h1T_sb = persist.tile([P, nloc_pad], BF, tag="h1T")

        qrot = [1, 2, 3, 0]  # q0 blocks the engine; issue it last per round
        qctr = [0]

        def next_q():
            q = qrot[qctr[0] % 4]
            qctr[0] += 1
            return q

        # ---------------- Phase A: layer-1 projection of own rows ----------
        nc.sync.dma_start_transpose(xT_sb[:, :], x16_d[:, :])
        for b in range(nb):
            sl = slice(b * P, (b + 1) * P)
            p_ps = psum_mm.tile([P, D], dt.float32, tag="mm")
            nc.tensor.matmul(p_ps[:], lhsT=xT_sb[:, sl], rhs=wsb["Wp1T"][:],
                             start=True, stop=not has_bias)
            if has_bias:
                nc.tensor.matmul(p_ps[:], lhsT=ones1[:], rhs=bp1r[:],
                                 start=False, stop=True)
            pr = outp.tile([P, D], F8, tag="pr")
            nc.scalar.activation(pr[:], p_ps[:], AF.Relu)
            nc.sync.dma_start(h1own[sl, :], pr[:])

        nc.gpsimd.collective_compute(
            "AllGather", OP.bypass, replica_groups=groups_all,
            ins=[h1own[:, :]], outs=[table1[:, :]],
        )

        # ---------------- message+aggregate for one layer -------------------
        def agg_layer(table, WlT, WrT, layer):
            tview = table[:, :].rearrange("(a b) d -> a (b d)", b=2)  # [npair, 256]
            cols = G * KT * P // 16
            nchunks = G * KT
            # chunk-aligned split of each group's gather into 4 calls (one per
            # SWDGE queue) for finer pipeline granularity
            splits = [0, nchunks // 4, nchunks // 2, 3 * nchunks // 4, nchunks]
            for g in range(ngroups):
                st = stage_p.tile([P, G * KT, 2 * D], F8, tag="st")
                for s0, s1 in zip(splits[:-1], splits[1:]):
                    n = (s1 - s0) * P
                    nc.gpsimd.dma_gather(
                        st[:, s0:s1, :], tview,
                        idx_sb[:, g * cols + s0 * P // 16 : g * cols + s1 * P // 16],
                        n, n, 2 * D, single_packet=False,
                        queue_num=next_q(),
                    )
                for bb in range(G):
                    b = g * G + bb
                    sl = slice(b * P, (b + 1) * P)
                    oh = ohp.tile([P, KT, P], F8, tag="oh")
                    nc.vector.tensor_tensor(
                        out=oh[:],
                        in0=dloc_sb[:, b * KT : (b + 1) * KT].unsqueeze(2).to_broadcast([P, KT, P]),
                        in1=iota[:].rearrange("p (k q) -> p k q", k=KT),
                        op=OP.is_equal,
                    )
                    # fp8 DoubleRow over consecutive chunk pairs; the parity
                    # bridge pair (last even, first odd) uses a 384-elem stride
                    base_ap = st[:]
                    pstride = base_ap.ap[0][0]

                    def chunk_addr(i):
                        return (bb * KT + i) * 2 * D + (0 if i < K0 else D)

                    def pair_lhsT(i):
                        stride = chunk_addr(i + 1) - chunk_addr(i)
                        return bass.AP(
                            base_ap.tensor,
                            base_ap.offset + chunk_addr(i),
                            [[pstride, P], [stride, 2], [1, D]],
                        )

                    ops = []
                    t = 0
                    while t + 2 <= KT:
                        ops.append((pair_lhsT(t), oh[:, t : t + 2, :], DR))
                        t += 2
                    if t < KT:
                        par = 0 if t < K0 else 1
                        ops.append((
                            st[:, bb * KT + t, par * D : (par + 1) * D],
                            oh[:, t, :],
                            None,
                        ))
                    agg_ps = psum_agg.tile([P, P], dt.float32)
                    for i, (l, r, pm) in enumerate(ops):
                        nc.tensor.matmul(
                            agg_ps[:], lhsT=l, rhs=r,
                            start=(i == 0), stop=(i == len(ops) - 1),
                            perf_mode=pm,
                        )
                    aggT = aggsb.tile([P, P], BF)
                    nc.vector.tensor_tensor(
                        out=aggT[:], in0=agg_ps[:], in1=invd_sb[:, sl], op=OP.mult
                    )
                    if layer == 1:
                        o_ps = psum_mm.tile([P, P], dt.float32, tag="mm")
                        nc.tensor.matmul(o_ps[:], lhsT=WlT[:], rhs=aggT[:], start=True, stop=False)
                        nc.tensor.matmul(o_ps[:], lhsT=WrT[:], rhs=xT_sb[:, sl], start=False, stop=True)
                        if has_bias:
                            nc.scalar.activation(h1T_sb[:, sl], o_ps[:], AF.Relu, bias=bl1c[:], scale=1.0)
                        else:
                            nc.scalar.activation(h1T_sb[:, sl], o_ps[:], AF.Relu)
                        # fused layer-2 projection of this block
                        p_ps = psum_mm.tile([P, D], dt.float32, tag="mm")
                        nc.tensor.matmul(p_ps[:], lhsT=h1T_sb[:, sl], rhs=wsb["Wp2T"][:],
                                         start=True, stop=not has_bias)
                        if has_bias:
                            nc.tensor.matmul(p_ps[:], lhsT=ones1[:], rhs=bp2r[:],
                                             start=False, stop=True)
                        pr = outp.tile([P, D], F8, tag="pr")
                        nc.scalar.activation(pr[:], p_ps[:], AF.Relu)
                        nc.sync.dma_start(h2own[sl, :], pr[:])
                    else:
                        o_ps = psum_mm.tile([P, D], dt.float32, tag="mm")
                        nc.tensor.matmul(o_ps[:], lhsT=aggT[:], rhs=WlT[:], start=True, stop=False)
                        nc.tensor.matmul(o_ps[:], lhsT=h1T_sb[:, sl], rhs=WrT[:],
                                         start=False, stop=not has_bias)
                        if has_bias:
                            nc.tensor.matmul(o_ps[:], lhsT=ones1[:], rhs=bl2r[:],
                                             start=False, stop=True)
                        ob = outp.tile([P, D], dt.float32, tag="ob")
                        nc.scalar.activation(ob[:], o_ps[:], AF.Copy)
                        nc.sync.dma_start(out_own[sl, :], ob[:])

        agg_layer(table1, wsb["Wl1T"], wsb["Wr1T"], layer=1)

        nc.gpsimd.collective_compute(
            "AllGather", OP.bypass, replica_groups=groups_all,
            ins=[h2own[:, :]], outs=[table2[:, :]],
        )

        agg_layer(table2, wsb["Wl2T"], wsb["Wr2T"], layer=2)

    nc.compile()
    return nc


def make_in_maps(inputs, per_core, n_nodes, cores, K0, K1, has_bias):
    nloc, nb, nloc_pad, npad, npair = _plan(n_nodes, cores)
    KT = K0 + K1
    x = np.asarray(inputs["x"], dtype=np.float32)
    consts = dict(
        Wp1T=np.asarray(inputs["Wp1"]).T.astype(NP_BF16),
        Wl1T=np.asarray(inputs["Wl1"]).T.astype(NP_BF16),
        Wr1T=np.asarray(inputs["Wr1"]).T.astype(NP_BF16),
        Wp2T=np.asarray(inputs["Wp2"]).T.astype(NP_BF16),
        Wl2T=np.asarray(inputs["Wl2"]).T.astype(NP_BF16),
        Wr2T=np.asarray(inputs["Wr2"]).T.astype(NP_BF16),
        iota=np.tile(np.arange(P, dtype=np.uint8)[None, :], (P, KT)).copy(),
    )
    if has_bias:
        consts.update(
            ones1=np.ones((1, D), dtype=np.float32).astype(NP_BF16),
            bp1r=np.asarray(inputs["bp1"], np.float32).reshape(1, D).astype(NP_BF16),
            bl1c=np.asarray(inputs["bl1"], np.float32).reshape(P, 1).copy(),
            bp2r=np.asarray(inputs["bp2"], np.float32).reshape(1, D).astype(NP_BF16),
            bl2r=np.asarray(inputs["bl2"], np.float32).reshape(1, D).astype(NP_BF16),
        )
    in_maps = []
    for c in range(cores):
        xo = np.zeros((nloc_pad, D), dtype=np.float32)
        xo[:nloc] = x[c * nloc : (c + 1) * nloc]
        m = dict(consts)
        m["x16"] = xo.astype(NP_BF16)
        m.update(per_core[c])
        in_maps.append(m)
    return in_maps


_BUILT = {}


def _run(inputs, n_nodes, n_edges, cores, G=7, trace=False):
    per_core, K0, K1, _ = preprocess(inputs["edge_index"], n_nodes, cores)
    has_bias = any(
        np.any(np.asarray(inputs[k]) != 0) for k in ["bp1", "bl1", "bp2", "bl2"]
    )
    key = (n_nodes, cores, K0, K1, G, has_bias)
    if key not in _BUILT:
        _BUILT[key] = build_nc(n_nodes, cores, K0, K1, G, has_bias)
    nc = _BUILT[key]
    in_maps = make_in_maps(inputs, per_core, n_nodes, cores, K0, K1, has_bias)
    res = run_bass_kernel_spmd(nc, in_maps, list(range(cores)), trace=trace)
    nloc, nb, nloc_pad, npad, npair = _plan(n_nodes, cores)
    out = np.concatenate([res.results[c]["out_own"][:nloc] for c in range(cores)], axis=0)
    return out.astype(np.float32), res


def kernel(**inputs):
    out, _ = _run(inputs, N_NODES, N_EDGES, CORES, G=7)
    return out
